# revision 5
# baseline (speedup 1.0000x reference)
"""DeepSpeed-style MLP (gelu-tanh MLP) on 8 TRN2 NeuronCores.

    out = gelu_tanh(input @ inter_w + inter_b) @ output_w + output_b
    input [4, 2048, 4096], inter_w [4096, 16384], output_w [16384, 4096]

Sharding: pure data-parallel over the flattened 8192 rows (1024 rows per
core); every core holds the full weights.  No collectives needed.  Each
core runs two chained GEMMs at fp32r precision (full PE rate for fp32
data), with the intermediate activation kept in transposed [F, M] layout
in an HBM scratch buffer so neither GEMM needs an activation transpose:

  GEMM1: H^T[f, m] = W1tile[k, f].T @ X^T[k, m]   (X^T built once via PE
         transposes), gelu+bias fused on the ScalarEngine on the way out
         of PSUM (bias is per-f = per-partition in this layout).
  GEMM2: OUT[m, d]  = H^T[f, m].T @ W2[f, d], accumulated over f-blocks
         into an SBUF accumulator so W2 streams from HBM exactly once.
"""

import os
import sys

import numpy as np

for _p in (
    "/root/.axon_site",
    "/root/.axon_site/_ro/trn_rl_repo",
    "/root/.axon_site/_ro/pypackages",
    "/opt/trn_rl_repo",
):
    if os.path.isdir(_p) and _p not in sys.path:
        sys.path.append(_p)

import concourse.bass as bass
import concourse.mybir as mybir
from concourse import bacc
from concourse.bass_utils import run_bass_kernel_spmd
from concourse.masks import make_identity
from concourse.tile import TileContext

P = 128
FP32 = mybir.dt.float32
FP32R = mybir.dt.float32r
GELU_TANH = mybir.ActivationFunctionType.Gelu_apprx_tanh
ADD = mybir.AluOpType.add

N_CORES = 8
B, S, D, F = 4, 2048, 4096, 16384
M_CORE = (B * S) // N_CORES  # 1024 rows per core


def build_mlp(M, D, F, N_M=None, F_BLK1=256, KCH=4, F_BLK2=512, D_SL=None, GSZ=None):
    """Build the per-core Bass program for out = gelu(x@w1+b1)@w2 + b2."""
    N_M = N_M or min(512, M)  # GEMM1 moving width (rows)
    D_SL = D_SL or min(512, D)  # GEMM2 moving width (output cols)
    NM = M // P  # 128-row blocks per core
    GSZ = GSZ or min(4, NM)  # psum group size in GEMM2
    KD = D // P  # contraction tiles for GEMM1
    KF = F // P  # f tiles total
    FS1 = F_BLK1 // P
    FI2 = F_BLK2 // P
    M_SL = M // N_M
    DS = D // D_SL
    assert D % P == 0 and F % P == 0 and M % P == 0
    assert F % F_BLK1 == 0 and F % F_BLK2 == 0 and KD % KCH == 0
    assert NM % GSZ == 0 and M % N_M == 0 and D % D_SL == 0

    nc = bacc.Bacc()
    x = nc.dram_tensor("x", (M, D), FP32, kind="ExternalInput")
    w1 = nc.dram_tensor("w1", (D, F), FP32R, kind="ExternalInput")
    b1 = nc.dram_tensor("b1", (F,), FP32, kind="ExternalInput")
    w2 = nc.dram_tensor("w2", (F, D), FP32R, kind="ExternalInput")
    b2 = nc.dram_tensor("b2", (D,), FP32, kind="ExternalInput")
    out = nc.dram_tensor("out", (M, D), FP32, kind="ExternalOutput")

    with TileContext(nc) as tc:
        with tc.tile_pool(name="dram", bufs=1, space="DRAM") as dram_pool:
            ht_dram = dram_pool.tile([F, M], FP32R, name="ht_scratch")

            # ---------------- phase 0: X^T, phase 1: GEMM1 ----------------
            with (
                tc.tile_pool(name="xt", bufs=1) as xt_pool,
                tc.tile_pool(name="xrow", bufs=2) as xrow_pool,
                tc.tile_pool(name="w1s", bufs=3) as w1_pool,
                tc.tile_pool(name="hstage", bufs=4) as hst_pool,
                tc.tile_pool(name="consts1", bufs=1) as const_pool,
            ):
                ident = const_pool.tile([P, P], FP32, name="ident")
                make_identity(nc, ident)
                b1_sb = const_pool.tile([P, KF], FP32, name="b1_sb")
                nc.sync.dma_start(b1_sb, b1[:].rearrange("(o p) -> p o", p=P))

                xt_tiles = [
                    xt_pool.tile([P, M], FP32R, name=f"xt{k}", tag=f"xt{k}") for k in range(KD)
                ]
                with tc.tile_pool(name="pst", bufs=4, space="PSUM") as pst_pool:
                    for mi in range(NM):
                        xrow = xrow_pool.tile([P, D], FP32, tag="xrow")
                        nc.sync.dma_start(xrow, x[mi * P : (mi + 1) * P, :])
                        for k in range(KD):
                            ps = pst_pool.tile([P, P], FP32, tag="tp")
                            nc.tensor.transpose(
                                ps, xrow[:, k * P : (k + 1) * P], ident
                            )
                            nc.vector.tensor_copy(
                                xt_tiles[k][:, mi * P : (mi + 1) * P], ps
                            )

                tc.strict_bb_all_engine_barrier()
                with tc.tile_pool(name="ps1", bufs=2, space="PSUM") as ps1_pool:
                    for fb in range(F // F_BLK1):
                        psums = [
                            [
                                ps1_pool.tile([P, N_M], FP32, tag=f"ps{f}_{m}", name=f"ps{f}_{m}")
                                for m in range(M_SL)
                            ]
                            for f in range(FS1)
                        ]
                        for kc in range(KD // KCH):
                            w1t = w1_pool.tile([P, KCH, F_BLK1], FP32R, tag="w1t")
                            nc.sync.dma_start(
                                w1t,
                                w1[:].rearrange("(ko p) f -> p ko f", p=P)[
                                    :,
                                    kc * KCH : (kc + 1) * KCH,
                                    fb * F_BLK1 : (fb + 1) * F_BLK1,
                                ],
                            )
                            for kk in range(KCH):
                                k = kc * KCH + kk
                                for f in range(FS1):
                                    for m in range(M_SL):
                                        nc.tensor.matmul(
                                            psums[f][m],
                                            lhsT=w1t[
                                                :, kk, f * P : (f + 1) * P
                                            ],
                                            rhs=xt_tiles[k][
                                                :, m * N_M : (m + 1) * N_M
                                            ],
                                            start=(k == 0),
                                            stop=(k == KD - 1),
                                        )
                        for f in range(FS1):
                            fglob = fb * FS1 + f
                            for m in range(M_SL):
                                hst = hst_pool.tile([P, N_M], FP32R, tag="hst")
                                nc.scalar.activation(
                                    hst,
                                    psums[f][m],
                                    GELU_TANH,
                                    bias=b1_sb[:, fglob : fglob + 1],
                                    scale=1.0,
                                )
                                nc.sync.dma_start(
                                    ht_dram[
                                        fglob * P : (fglob + 1) * P,
                                        m * N_M : (m + 1) * N_M,
                                    ],
                                    hst,
                                )

            # ---------------- phase 2: GEMM2 with SBUF accumulator --------
            tc.strict_bb_all_engine_barrier()
            with (
                tc.tile_pool(name="acc", bufs=1) as acc_pool,
                tc.tile_pool(name="htp", bufs=2) as htp_pool,
                tc.tile_pool(name="w2s", bufs=2) as w2_pool,
                tc.tile_pool(name="consts2", bufs=1) as const2_pool,
                tc.tile_pool(name="ps2", bufs=2, space="PSUM") as ps2_pool,
            ):
                b2_sb = const2_pool.tile([P, D], FP32, name="b2_sb")
                nc.gpsimd.dma_start(
                    out=b2_sb,
                    in_=bass.AP(tensor=b2[:].tensor, offset=0, ap=[[0, P], [1, D]]),
                )
                acc = [
                    acc_pool.tile([P, D], FP32, name=f"acc{i}", tag=f"acc{i}") for i in range(NM)
                ]
                FB2 = F // F_BLK2
                for fb2 in range(FB2):
                    htp = [
                        htp_pool.tile([P, M], FP32R, tag=f"htp{i}", name=f"htp{i}")
                        for i in range(FI2)
                    ]
                    for i in range(FI2):
                        fg = fb2 * FI2 + i
                        nc.sync.dma_start(htp[i], ht_dram[fg * P : (fg + 1) * P, :])
                    for ds in range(DS):
                        w2ts = [
                            w2_pool.tile([P, D_SL], FP32R, tag=f"w2t{i}", name=f"w2t{i}")
                            for i in range(FI2)
                        ]
                        for i in range(FI2):
                            fg = fb2 * FI2 + i
                            nc.sync.dma_start(
                                w2ts[i],
                                w2[
                                    fg * P : (fg + 1) * P,
                                    ds * D_SL : (ds + 1) * D_SL,
                                ],
                            )
                        for g in range(NM // GSZ):
                            pss = [
                                ps2_pool.tile([P, D_SL], FP32, tag=f"ps2_{j}", name=f"ps2_{j}")
                                for j in range(GSZ)
                            ]
                            for i in range(FI2):
                                for j in range(GSZ):
                                    msub = g * GSZ + j
                                    nc.tensor.matmul(
                                        pss[j],
                                        lhsT=htp[i][
                                            :, msub * P : (msub + 1) * P
                                        ],
                                        rhs=w2ts[i],
                                        start=(i == 0),
                                        stop=(i == FI2 - 1),
                                    )
                            for j in range(GSZ):
                                msub = g * GSZ + j
                                a_sl = acc[msub][:, ds * D_SL : (ds + 1) * D_SL]
                                if fb2 == 0:
                                    nc.vector.tensor_tensor(
                                        a_sl,
                                        pss[j],
                                        b2_sb[:, ds * D_SL : (ds + 1) * D_SL],
                                        ADD,
                                    )
                                else:
                                    nc.vector.tensor_add(a_sl, a_sl, pss[j])
                for msub in range(NM):
                    nc.sync.dma_start(out[msub * P : (msub + 1) * P, :], acc[msub])

    nc.finalize()
    return nc


_BUILT = {}


def _get_program(M, D, F):
    key = (M, D, F)
    if key not in _BUILT:
        _BUILT[key] = build_mlp(M, D, F)
    return _BUILT[key]


def run(inputs, trace=False, M=None, D_=None, F_=None):
    """Run the SPMD kernel on 8 cores. Returns (out[rows, D], BassKernelResults)."""
    x = np.ascontiguousarray(np.asarray(inputs["input"], dtype=np.float32))
    w1 = np.ascontiguousarray(np.asarray(inputs["inter_w"], dtype=np.float32))
    b1 = np.ascontiguousarray(np.asarray(inputs["inter_b"], dtype=np.float32))
    w2 = np.ascontiguousarray(np.asarray(inputs["output_w"], dtype=np.float32))
    b2 = np.ascontiguousarray(np.asarray(inputs["output_b"], dtype=np.float32))

    d = w1.shape[0]
    f = w1.shape[1]
    xf = x.reshape(-1, d)
    rows = xf.shape[0]
    m_core = rows // N_CORES
    nc = _get_program(m_core, d, f)

    in_maps = []
    for c in range(N_CORES):
        in_maps.append(
            {
                "x": np.ascontiguousarray(xf[c * m_core : (c + 1) * m_core]),
                "w1": w1,
                "b1": b1,
                "w2": w2,
                "b2": b2,
            }
        )
    res = run_bass_kernel_spmd(
        nc, in_maps, core_ids=list(range(N_CORES)), trace=trace
    )
    outf = np.concatenate([res.results[c]["out"] for c in range(N_CORES)], axis=0)
    return outf, res


def kernel(input, inter_w, inter_b, output_w, output_b):
    inputs = {
        "input": input,
        "inter_w": inter_w,
        "inter_b": inter_b,
        "output_w": output_w,
        "output_b": output_b,
    }
    outf, _ = run(inputs, trace=False)
    return outf.reshape(np.asarray(input).shape[:-1] + (outf.shape[-1],)).astype(
        np.float32
    )


# revision 7
# speedup vs baseline: 1.0092x; 1.0092x over previous
"""DeepSpeed-style MLP (gelu-tanh MLP) on 8 TRN2 NeuronCores.

    out = gelu_tanh(input @ inter_w + inter_b) @ output_w + output_b
    input [4, 2048, 4096], inter_w [4096, 16384], output_w [16384, 4096]

Sharding: pure data-parallel over the flattened 8192 rows (1024 rows per
core); every core holds the full weights.  No collectives needed.  Each
core runs two chained GEMMs at fp32r precision (full PE rate for fp32
data), with the intermediate activation kept in transposed [F, M] layout
in an HBM scratch buffer so neither GEMM needs an activation transpose:

  GEMM1: H^T[f, m] = W1tile[k, f].T @ X^T[k, m]   (X^T built once via PE
         transposes), gelu+bias fused on the ScalarEngine on the way out
         of PSUM (bias is per-f = per-partition in this layout).
  GEMM2: OUT[m, d]  = H^T[f, m].T @ W2[f, d], accumulated over f-blocks
         into an SBUF accumulator so W2 streams from HBM exactly once.
"""

import os
import sys

import numpy as np

for _p in (
    "/root/.axon_site",
    "/root/.axon_site/_ro/trn_rl_repo",
    "/root/.axon_site/_ro/pypackages",
    "/opt/trn_rl_repo",
):
    if os.path.isdir(_p) and _p not in sys.path:
        sys.path.append(_p)

import concourse.bass as bass
import concourse.mybir as mybir
from concourse import bacc
from concourse.bass_utils import run_bass_kernel_spmd
from concourse.masks import make_identity
from concourse.tile import TileContext

P = 128
FP32 = mybir.dt.float32
FP32R = mybir.dt.float32r
GELU_TANH = mybir.ActivationFunctionType.Gelu_apprx_tanh
ADD = mybir.AluOpType.add

N_CORES = 8
B, S, D, F = 4, 2048, 4096, 16384
M_CORE = (B * S) // N_CORES  # 1024 rows per core


def build_mlp(M, D, F, N_M=None, F_BLK1=256, KCH=4, F_BLK2=512, D_SL=None, GSZ=None):
    """Build the per-core Bass program for out = gelu(x@w1+b1)@w2 + b2."""
    N_M = N_M or min(512, M)  # GEMM1 moving width (rows)
    D_SL = D_SL or min(512, D)  # GEMM2 moving width (output cols)
    NM = M // P  # 128-row blocks per core
    GSZ = GSZ or min(4, NM)  # psum group size in GEMM2
    KD = D // P  # contraction tiles for GEMM1
    KF = F // P  # f tiles total
    FS1 = F_BLK1 // P
    FI2 = F_BLK2 // P
    M_SL = M // N_M
    DS = D // D_SL
    assert D % P == 0 and F % P == 0 and M % P == 0
    assert F % F_BLK1 == 0 and F % F_BLK2 == 0 and KD % KCH == 0
    assert NM % GSZ == 0 and M % N_M == 0 and D % D_SL == 0

    nc = bacc.Bacc()
    x = nc.dram_tensor("x", (M, D), FP32, kind="ExternalInput")
    w1 = nc.dram_tensor("w1", (D, F), FP32R, kind="ExternalInput")
    b1 = nc.dram_tensor("b1", (F,), FP32, kind="ExternalInput")
    w2 = nc.dram_tensor("w2", (F, D), FP32R, kind="ExternalInput")
    b2 = nc.dram_tensor("b2", (D,), FP32, kind="ExternalInput")
    out = nc.dram_tensor("out", (M, D), FP32, kind="ExternalOutput")

    with TileContext(nc) as tc:
        with tc.tile_pool(name="dram", bufs=1, space="DRAM") as dram_pool:
            NCH = next(
                n for n in (4, 2, 1) if F % (n * F_BLK2) == 0 and F % (n * F_BLK1) == 0
            )
            FCH = F // NCH
            ht_chunks = [
                dram_pool.tile([FCH, M], FP32R, name=f"ht_scratch{i}")
                for i in range(NCH)
            ]

            def ht_slice(fglob_row0, nrows, c0, c1):
                ch = (fglob_row0 * P) // FCH
                r0 = fglob_row0 * P - ch * FCH
                return ht_chunks[ch][r0 : r0 + nrows, c0:c1]

            # ---------------- phase 0: X^T, phase 1: GEMM1 ----------------
            with (
                tc.tile_pool(name="xt", bufs=1) as xt_pool,
                tc.tile_pool(name="xrow", bufs=2) as xrow_pool,
                tc.tile_pool(name="w1s", bufs=3) as w1_pool,
                tc.tile_pool(name="hstage", bufs=4) as hst_pool,
                tc.tile_pool(name="consts1", bufs=1) as const_pool,
            ):
                ident = const_pool.tile([P, P], FP32, name="ident")
                make_identity(nc, ident)
                b1_sb = const_pool.tile([P, KF], FP32, name="b1_sb")
                nc.sync.dma_start(b1_sb, b1[:].rearrange("(o p) -> p o", p=P))

                xt_tiles = [
                    [
                        xt_pool.tile(
                            [P, N_M], FP32R, name=f"xt{k}_{m}", tag=f"xt{k}_{m}"
                        )
                        for m in range(M_SL)
                    ]
                    for k in range(KD)
                ]
                with tc.tile_pool(name="pst", bufs=4, space="PSUM") as pst_pool:
                    for mi in range(NM):
                        xrow = xrow_pool.tile([P, D], FP32, tag="xrow")
                        nc.sync.dma_start(xrow, x[mi * P : (mi + 1) * P, :])
                        for k in range(KD):
                            ps = pst_pool.tile([P, P], FP32, tag="tp")
                            nc.tensor.transpose(
                                ps, xrow[:, k * P : (k + 1) * P], ident
                            )
                            nc.vector.tensor_copy(
                                xt_tiles[k][(mi * P) // N_M][
                                    :, mi * P - ((mi * P) // N_M) * N_M :
                                    mi * P - ((mi * P) // N_M) * N_M + P
                                ],
                                ps,
                            )

                with tc.tile_pool(name="ps1", bufs=2, space="PSUM") as ps1_pool:
                    for fb in range(F // F_BLK1):
                        psums = [
                            [
                                ps1_pool.tile([P, N_M], FP32, tag=f"ps{f}_{m}", name=f"ps{f}_{m}")
                                for m in range(M_SL)
                            ]
                            for f in range(FS1)
                        ]
                        for kc in range(KD // KCH):
                            w1t = w1_pool.tile([P, KCH, F_BLK1], FP32R, tag="w1t")
                            nc.sync.dma_start(
                                w1t,
                                w1[:].rearrange("(ko p) f -> p ko f", p=P)[
                                    :,
                                    kc * KCH : (kc + 1) * KCH,
                                    fb * F_BLK1 : (fb + 1) * F_BLK1,
                                ],
                            )
                            for kk in range(KCH):
                                k = kc * KCH + kk
                                for f in range(FS1):
                                    for m in range(M_SL):
                                        nc.tensor.matmul(
                                            psums[f][m],
                                            lhsT=w1t[
                                                :, kk, f * P : (f + 1) * P
                                            ],
                                            rhs=xt_tiles[k][m][:],
                                            start=(k == 0),
                                            stop=(k == KD - 1),
                                        )
                        for f in range(FS1):
                            fglob = fb * FS1 + f
                            for m in range(M_SL):
                                hst = hst_pool.tile([P, N_M], FP32R, tag="hst")
                                nc.scalar.activation(
                                    hst,
                                    psums[f][m],
                                    GELU_TANH,
                                    bias=b1_sb[:, fglob : fglob + 1],
                                    scale=1.0,
                                )
                                nc.sync.dma_start(
                                    ht_slice(fglob, P, m * N_M, (m + 1) * N_M),
                                    hst,
                                )

            # ---------------- phase 2: GEMM2 with SBUF accumulator --------
            with (
                tc.tile_pool(name="acc", bufs=1) as acc_pool,
                tc.tile_pool(name="htp", bufs=2) as htp_pool,
                tc.tile_pool(name="w2s", bufs=2) as w2_pool,
                tc.tile_pool(name="consts2", bufs=1) as const2_pool,
                tc.tile_pool(name="ps2", bufs=2, space="PSUM") as ps2_pool,
            ):
                b2_sb = const2_pool.tile([P, D], FP32, name="b2_sb")
                nc.gpsimd.dma_start(
                    out=b2_sb,
                    in_=bass.AP(tensor=b2[:].tensor, offset=0, ap=[[0, P], [1, D]]),
                )
                acc = [
                    acc_pool.tile([P, D], FP32, name=f"acc{i}", tag=f"acc{i}") for i in range(NM)
                ]
                FB2 = F // F_BLK2
                for fb2 in range(FB2):
                    htp = [
                        htp_pool.tile([P, M], FP32R, tag=f"htp{i}", name=f"htp{i}")
                        for i in range(FI2)
                    ]
                    for i in range(FI2):
                        fg = fb2 * FI2 + i
                        nc.sync.dma_start(htp[i], ht_slice(fg, P, 0, M))
                    for ds in range(DS):
                        w2ts = [
                            w2_pool.tile([P, D_SL], FP32R, tag=f"w2t{i}", name=f"w2t{i}")
                            for i in range(FI2)
                        ]
                        for i in range(FI2):
                            fg = fb2 * FI2 + i
                            nc.sync.dma_start(
                                w2ts[i],
                                w2[
                                    fg * P : (fg + 1) * P,
                                    ds * D_SL : (ds + 1) * D_SL,
                                ],
                            )
                        for g in range(NM // GSZ):
                            pss = [
                                ps2_pool.tile([P, D_SL], FP32, tag=f"ps2_{j}", name=f"ps2_{j}")
                                for j in range(GSZ)
                            ]
                            for i in range(FI2):
                                for j in range(GSZ):
                                    msub = g * GSZ + j
                                    nc.tensor.matmul(
                                        pss[j],
                                        lhsT=htp[i][
                                            :, msub * P : (msub + 1) * P
                                        ],
                                        rhs=w2ts[i],
                                        start=(i == 0),
                                        stop=(i == FI2 - 1),
                                    )
                            for j in range(GSZ):
                                msub = g * GSZ + j
                                a_sl = acc[msub][:, ds * D_SL : (ds + 1) * D_SL]
                                if fb2 == 0:
                                    nc.vector.tensor_tensor(
                                        a_sl,
                                        pss[j],
                                        b2_sb[:, ds * D_SL : (ds + 1) * D_SL],
                                        ADD,
                                    )
                                else:
                                    nc.vector.tensor_add(a_sl, a_sl, pss[j])
                                if fb2 == FB2 - 1:
                                    nc.sync.dma_start(
                                        out[
                                            msub * P : (msub + 1) * P,
                                            ds * D_SL : (ds + 1) * D_SL,
                                        ],
                                        a_sl,
                                    )

    nc.finalize()
    return nc


_BUILT = {}


def _get_program(M, D, F):
    key = (M, D, F)
    if key not in _BUILT:
        _BUILT[key] = build_mlp(M, D, F)
    return _BUILT[key]


def run(inputs, trace=False, M=None, D_=None, F_=None):
    """Run the SPMD kernel on 8 cores. Returns (out[rows, D], BassKernelResults)."""
    x = np.ascontiguousarray(np.asarray(inputs["input"], dtype=np.float32))
    w1 = np.ascontiguousarray(np.asarray(inputs["inter_w"], dtype=np.float32))
    b1 = np.ascontiguousarray(np.asarray(inputs["inter_b"], dtype=np.float32))
    w2 = np.ascontiguousarray(np.asarray(inputs["output_w"], dtype=np.float32))
    b2 = np.ascontiguousarray(np.asarray(inputs["output_b"], dtype=np.float32))

    d = w1.shape[0]
    f = w1.shape[1]
    xf = x.reshape(-1, d)
    rows = xf.shape[0]
    m_core = rows // N_CORES
    nc = _get_program(m_core, d, f)

    in_maps = []
    for c in range(N_CORES):
        in_maps.append(
            {
                "x": np.ascontiguousarray(xf[c * m_core : (c + 1) * m_core]),
                "w1": w1,
                "b1": b1,
                "w2": w2,
                "b2": b2,
            }
        )
    res = run_bass_kernel_spmd(
        nc, in_maps, core_ids=list(range(N_CORES)), trace=trace
    )
    outf = np.concatenate([res.results[c]["out"] for c in range(N_CORES)], axis=0)
    return outf, res


def kernel(input, inter_w, inter_b, output_w, output_b):
    inputs = {
        "input": input,
        "inter_w": inter_w,
        "inter_b": inter_b,
        "output_w": output_w,
        "output_b": output_b,
    }
    outf, _ = run(inputs, trace=False)
    return outf.reshape(np.asarray(input).shape[:-1] + (outf.shape[-1],)).astype(
        np.float32
    )
